# revision 26
# baseline (speedup 1.0000x reference)
"""CTC center-loss kernel for Trainium2, data-parallel over 8 NeuronCores.

Math (reference semantics):
    d         = features - centers[labels]
    loss      = 0.5 * sum(d*d)
    counts[c] = #{i : labels[i] == c}
    new_centers[c] = centers[c] + ALPHA * (S[c] - counts[c]*centers[c]) / (1 + counts[c])
  where S[c] = sum of features rows with label c.

Everything reduces to three streaming statistics over features:
    S [86,256], counts [86], sumsq = sum(features^2)
obtained with one-hot matmuls per 128-sample tile (contraction over the
sample/partition axis), accumulated in PSUM over all tiles:
    m1: acc  [86,256] += onehot.T @ features_tile
    m2: accc [86,1]   += onehot.T @ ones
    m3: accs [1,1]    += per_sample_sumsq.T @ ones
The per-core partials are AllReduce-summed across the 8 cores, then every
core applies the tiny [86,256] update and computes the scalar loss.
"""

import os
import sys

for _p in (
    "/root/.axon_site",
    "/root/.axon_site/_ro/trn_rl_repo",
    "/root/.axon_site/_ro/pypackages",
    "/opt/trn_rl_repo",
    "/opt/pypackages",
):
    if os.path.isdir(_p) and _p not in sys.path:
        sys.path.append(_p)

import numpy as np

ALPHA = 0.5
C = 86          # num classes
F = 256         # feature dim
N = 131072      # total samples
NCORES = 8
NS = N // NCORES          # samples per core = 16384
P = 128                   # partitions
TILES = NS // P           # 128 sample-tiles per core
T = 16                    # sample-tiles per super-tile (DMA batch)
NSUPER = TILES // T       # 8
FW = F + 2                # 258: payload row = [S_c | count_c | sumsq-slot]


def build_nc(feat_bufs=4, unroll=1, serialize=True, legalize=True):
    import concourse.bass as bass
    import concourse.mybir as mybir
    from concourse import tile
    from concourse.masks import make_identity

    f32 = mybir.dt.float32
    i32 = mybir.dt.int32

    nc = bass.Bass(
        "TRN2",
        target_bir_lowering=False,
        debug=False,
        num_devices=NCORES,
    )

    feat = nc.dram_tensor("features", [NS, F], f32, kind="ExternalInput").ap()
    lab = nc.dram_tensor("labels", [NS], i32, kind="ExternalInput").ap()
    cent = nc.dram_tensor("centers", [C, F], f32, kind="ExternalInput").ap()
    out_c = nc.dram_tensor("out_centers", [C, F], f32, kind="ExternalOutput").ap()
    out_l = nc.dram_tensor("out_loss", [1, 1], f32, kind="ExternalOutput").ap()

    with tile.TileContext(nc) as tc:
        with (
            tc.tile_pool(name="const", bufs=1) as const_pool,
            tc.tile_pool(name="lprep", bufs=1) as lprep_pool,
            tc.tile_pool(name="feat", bufs=feat_bufs) as feat_pool,
            tc.tile_pool(name="oh", bufs=12) as oh_pool,
            tc.tile_pool(name="sq", bufs=2) as sq_pool,
            tc.tile_pool(name="fin", bufs=1) as fin_pool,
            tc.tile_pool(name="psum", bufs=1, space="PSUM") as psum_pool,
            tc.tile_pool(name="psum2", bufs=1, space="PSUM") as psum2_pool,
            tc.tile_pool(name="dram", bufs=1, space="DRAM") as dram_pool,
        ):
            # ---- compile-time constants (shared across bench iterations) ----
            iota_i = const_pool.tile([P, C], i32)
            nc.gpsimd.iota(iota_i[:, :], pattern=[[1, C]], base=0,
                           channel_multiplier=0)
            iota_f = const_pool.tile([P, C], f32)
            nc.vector.tensor_copy(iota_f[:, :], iota_i[:, :])
            ones_col = const_pool.tile([P, 1], f32)
            nc.vector.memset(ones_col[:, :], 1.0)

            prev_cc_out = None
            for it in range(unroll):
                prev_cc_out = _one_pass(
                    nc, tc, mybir, feat, lab, cent, out_c, out_l,
                    iota_f, ones_col,
                    lprep_pool, feat_pool, oh_pool, sq_pool, fin_pool,
                    psum_pool, psum2_pool, dram_pool,
                    prev_cc_out if serialize else None)

    if legalize:
        _legalize(nc)
    return nc


def _legalize(nc):
    """Make the module digestible by this container's (older) walrus.

    1. Remove kernel-exit EVENT_SEMAPHORE_RANGE_CLEAR InstISA ops: walrus
       rejects their encoding ("ISA wrong length") and they only pre-clear
       semaphores for a subsequent bass kernel in the same NEFF, which we
       never emit.
    2. Engine instruction structs here accept a single inline sync wait
       ("Too many sync wait commands" otherwise). Hoist all but one wait of
       any multi-wait instruction onto wait-only EventSemaphore instructions
       issued just before it on the same engine.
    """
    from concourse import mybir

    removed = set()
    for f in nc.m.functions:
        for b in f.blocks:
            keep = []
            for inst in b.instructions:
                if type(inst).__name__ == "InstISA":
                    removed.add(inst.name)
                else:
                    keep.append(inst)
            b.instructions[:] = keep
    if removed:
        for f in nc.m.functions:
            for b in f.blocks:
                for inst in b.instructions:
                    deps = list(inst.sync_dependencies())
                    if any(d in removed for d, _ in deps):
                        inst.set_sync_dependencies(
                            [(d, i) for d, i in deps if d not in removed])

    for f in nc.m.functions:
        for b in f.blocks:
            out = []
            for inst in b.instructions:
                si = inst.sync_info
                if si is not None and si.on_wait and len(si.on_wait) > 1:
                    waits = list(si.on_wait)
                    for w in waits[:-1]:
                        ev = mybir.InstEventSemaphore(
                            name=f"{inst.name}_hw{w.id}", ins=[], outs=[])
                        ev.engine = inst.engine
                        ev.sync_info = mybir.SyncInfo(
                            on_wait=[w], on_update=[])
                        out.append(ev)
                    inst.sync_info = mybir.SyncInfo(
                        on_wait=[waits[-1]], on_update=list(si.on_update))
                out.append(inst)
            b.instructions[:] = out


def _one_pass(nc, tc, mybir, feat, lab, cent, out_c, out_l,
              iota_f, ones_col,
              lprep_pool, feat_pool, oh_pool, sq_pool, fin_pool,
              psum_pool, psum2_pool, dram_pool, prev_cc_out):
    f32 = mybir.dt.float32
    i32 = mybir.dt.int32

    # ---- label prep ----
    # Samples are remapped so partition p of supertile s holds the 16
    # CONSECUTIVE rows s*2048 + p*16 .. +15 (16KB contiguous HBM runs for
    # the feature DMA). labT[:, s*16+t] holds matmul-tile (s,t)'s labels;
    # the (s p t) rearrange makes this a pure strided DMA - no transpose.
    lab_i = lprep_pool.tile([P, TILES], i32)
    nc.sync.dma_start(
        out=lab_i[:, :].rearrange("p (s t) -> p s t", t=T),
        in_=lab.rearrange("(s p t) -> p s t", p=P, t=T))
    labT = lprep_pool.tile([P, TILES], f32)
    nc.vector.tensor_copy(labT[:, :], lab_i[:, :])

    # centers for the finalize stage
    ct = lprep_pool.tile([C, F], f32)
    nc.sync.dma_start(out=ct[:, :], in_=cent)

    # ---- streaming one-hot matmul accumulation ----
    # feature tiles carry a ones column (memset once per supertile on the
    # idle Pool engine): the single matmul per tile then yields both the
    # per-class feature sums AND the counts in psum [86, 257]. The
    # supertile-wide ACT square accumulates sum(f^2) + 16 (ones cols) per
    # partition; the spurious +N is subtracted at finalize.
    FTW = F + 1
    acc = psum_pool.tile([C, FTW], f32, tag="acc")
    sqbuf = lprep_pool.tile([P, NSUPER], f32)

    for s in range(NSUPER):
        ft = feat_pool.tile([P, T * FTW], f32)
        if prev_cc_out is not None:
            # benchmarking only: chain this pass's loads on the previous
            # pass's collective so passes don't overlap
            flat = prev_cc_out[:, :].rearrange("c f -> (c f)")
            nc.sync.dma_start(
                out=ft[:, 0:1],
                in_=flat[s * P:(s + 1) * P].rearrange("(p o) -> p o", o=1))
        fv = ft[:, :].rearrange("p (t f) -> p t f", f=FTW)
        src = feat[s * T * P:(s + 1) * T * P, :].rearrange(
            "(p t) f -> p t f", t=T)
        nc.sync.dma_start(out=fv[:, :, 0:F], in_=src)
        nc.gpsimd.memset(fv[:, :, F:FTW], 1.0)
        sqo = sq_pool.tile([P, T * FTW], f32, tag="sqo")
        nc.scalar.activation(
            out=sqo[:, :], in_=ft[:, :],
            func=mybir.ActivationFunctionType.Square,
            accum_out=sqbuf[:, s:s + 1])
        for t in range(T):
            i = s * T + t
            oh = oh_pool.tile([P, C], f32)
            nc.vector.tensor_single_scalar(
                out=oh[:, :], in_=iota_f[:, :],
                scalar=labT[:, i:i + 1],
                op=mybir.AluOpType.is_equal)
            nc.tensor.matmul(
                acc[:, :], lhsT=oh[:, :], rhs=ft[:, t * FTW:(t + 1) * FTW],
                start=(i == 0), stop=(i == TILES - 1))

    # ---- reduce the sumsq accumulator with one matmul ----
    sqr = fin_pool.tile([P, 1], f32, tag="sqr")
    nc.vector.reduce_sum(sqr[:, :], sqbuf[:, :], axis=mybir.AxisListType.X)
    sq_ps = psum_pool.tile([1, 1], f32, tag="sq_ps")
    nc.tensor.matmul(sq_ps[:, :], lhsT=sqr[:, :], rhs=ones_col[:, :],
                     start=True, stop=True)

    # ---- pack partials [S | counts | sumsq@row0]; AllGather + local sum
    # (AG floor ~5us vs AR ~18us at this size) ----
    part = fin_pool.tile([C, FW], f32, tag="part")
    nc.vector.tensor_copy(part[:, 0:FTW], acc[:, :])
    nc.vector.memset(part[0:C, F + 1:FW], 0.0)
    nc.vector.tensor_copy(part[0:1, F + 1:FW], sq_ps[:, :])

    cc_in = dram_pool.tile([C, FW], f32)
    cc_out = dram_pool.tile([NCORES * C, FW], f32)
    nc.sync.dma_start(out=cc_in[:, :], in_=part[:, :])
    nc.gpsimd.collective_compute(
        "AllGather",
        mybir.AluOpType.bypass,
        replica_groups=[list(range(NCORES))],
        ins=[cc_in[:, :].opt()],
        outs=[cc_out[:, :].opt()],
    )
    agbuf = fin_pool.tile([C, NCORES * FW], f32, tag="agbuf")
    agv = agbuf[:, :].rearrange("c (r f) -> c r f", f=FW)
    nc.sync.dma_start(
        out=agv[:, :, :],
        in_=cc_out[:, :].rearrange("(r c) f -> c r f", c=C))
    allsum = fin_pool.tile([C, FW], f32, tag="allsum")
    nc.vector.tensor_add(allsum[:, :], agv[:, 0, :], agv[:, 1, :])
    for r in range(2, NCORES):
        nc.vector.tensor_add(allsum[:, :], allsum[:, :], agv[:, r, :])

    # ---- finalize: new_centers = a*centers + r*S ----
    S = allsum[0:C, 0:F]
    cnt = allsum[0:C, F:F + 1]
    sqt_t = allsum  # [0:1, F+1:FW] holds the global sum of squares

    cnt1 = fin_pool.tile([C, 1], f32, tag="cnt1")
    nc.vector.tensor_scalar_add(cnt1[:, :], cnt, 1.0)
    rec = fin_pool.tile([C, 1], f32, tag="rec")
    nc.vector.reciprocal(rec[:, :], cnt1[:, :])
    r = fin_pool.tile([C, 1], f32, tag="r")
    nc.vector.tensor_scalar_mul(r[:, :], rec[:, :], ALPHA)
    rcnt = fin_pool.tile([C, 1], f32, tag="rcnt")
    nc.vector.tensor_mul(rcnt[:, :], r[:, :], cnt)
    a = fin_pool.tile([C, 1], f32, tag="a")
    nc.vector.tensor_scalar(
        out=a[:, :], in0=rcnt[:, :], scalar1=-1.0, scalar2=1.0,
        op0=mybir.AluOpType.mult, op1=mybir.AluOpType.add)

    newc = fin_pool.tile([C, F], f32, tag="newc")
    nc.vector.tensor_single_scalar(
        out=newc[:, :], in_=ct[:, :], scalar=a[:, 0:1],
        op=mybir.AluOpType.mult)
    sterm = fin_pool.tile([C, F], f32, tag="sterm")
    nc.vector.tensor_single_scalar(
        out=sterm[:, :], in_=S, scalar=r[:, 0:1],
        op=mybir.AluOpType.mult)
    newc2 = fin_pool.tile([C, F], f32, tag="newc2")
    nc.vector.tensor_add(newc2[:, :], newc[:, :], sterm[:, :])
    nc.sync.dma_start(out=out_c, in_=newc2[:, :])

    # ---- loss = 0.5*sumsq - sum_c dot(S,c) + 0.5*sum_c cnt*|c|^2 ----
    g = fin_pool.tile([C, 1], f32, tag="g")
    dots = fin_pool.tile([C, 1], f32, tag="dots")
    scr1 = fin_pool.tile([C, F], f32, tag="scr1")
    nc.vector.tensor_mul(scr1[:, :], S, ct[:, :])
    nc.vector.reduce_sum(dots[:, :], scr1[:, :], axis=mybir.AxisListType.X)
    c2 = fin_pool.tile([C, 1], f32, tag="c2")
    scr2 = fin_pool.tile([C, F], f32, tag="scr2")
    nc.vector.tensor_mul(scr2[:, :], ct[:, :], ct[:, :])
    nc.vector.reduce_sum(c2[:, :], scr2[:, :], axis=mybir.AxisListType.X)
    h = fin_pool.tile([C, 1], f32, tag="h")
    nc.vector.tensor_mul(h[:, :], c2[:, :], cnt)
    hm = fin_pool.tile([C, 1], f32, tag="hm")
    nc.vector.tensor_scalar_mul(hm[:, :], h[:, :], 0.5)
    nc.vector.tensor_sub(g[:, :], hm[:, :], dots[:, :])

    ones_c1 = fin_pool.tile([C, 1], f32, tag="ones_c1")
    nc.vector.memset(ones_c1[:, :], 1.0)
    lps = psum2_pool.tile([1, 1], f32, tag="ps2")
    nc.tensor.matmul(lps[:, :], lhsT=g[:, :], rhs=ones_c1[:, :],
                     start=True, stop=True)
    sqh = fin_pool.tile([1, 1], f32, tag="sqh")
    nc.vector.tensor_scalar(
        out=sqh[:, :], in0=sqt_t[0:1, F + 1:FW], scalar1=-float(N),
        scalar2=0.5, op0=mybir.AluOpType.add, op1=mybir.AluOpType.mult)
    lsb = fin_pool.tile([1, 1], f32, tag="lsb")
    nc.vector.tensor_add(lsb[:, :], lps[0:1, 0:1], sqh[:, :])
    nc.sync.dma_start(out=out_l, in_=lsb[:, :])

    return cc_out


_NC_CACHE = None
LAST_RESULT = None  # stashed BassKernelResults for test.py introspection


def _get_nc():
    global _NC_CACHE
    if _NC_CACHE is None:
        _NC_CACHE = build_nc()
    return _NC_CACHE


def kernel(features, labels, centers):
    global LAST_RESULT
    from concourse.bass_utils import run_bass_kernel_spmd

    features = np.ascontiguousarray(np.asarray(features, dtype=np.float32))
    labels = np.asarray(labels).astype(np.int32).ravel()
    centers = np.ascontiguousarray(np.asarray(centers, dtype=np.float32))

    nc = _get_nc()
    in_maps = [
        {
            "features": features[r * NS:(r + 1) * NS],
            "labels": labels[r * NS:(r + 1) * NS],
            "centers": centers,
        }
        for r in range(NCORES)
    ]
    trace = os.environ.get("KERNEL_TRACE", "0") == "1"
    res = run_bass_kernel_spmd(
        nc, in_maps, core_ids=list(range(NCORES)), trace=trace)
    LAST_RESULT = res
    out = res.results[0]
    loss = np.float32(np.asarray(out["out_loss"]).reshape(-1)[0])
    new_centers = np.asarray(out["out_centers"], dtype=np.float32).reshape(C, F)
    return loss, new_centers


# revision 27
# speedup vs baseline: 1.1145x; 1.1145x over previous
"""CTC center-loss kernel for Trainium2, data-parallel over 8 NeuronCores.

Math (reference semantics):
    d         = features - centers[labels]
    loss      = 0.5 * sum(d*d)
    counts[c] = #{i : labels[i] == c}
    new_centers[c] = centers[c] + ALPHA * (S[c] - counts[c]*centers[c]) / (1 + counts[c])
  where S[c] = sum of features rows with label c.

Everything reduces to three streaming statistics over features:
    S [86,256], counts [86], sumsq = sum(features^2)
obtained with one-hot matmuls per 128-sample tile (contraction over the
sample/partition axis), accumulated in PSUM over all tiles:
    m1: acc  [86,256] += onehot.T @ features_tile
    m2: accc [86,1]   += onehot.T @ ones
    m3: accs [1,1]    += per_sample_sumsq.T @ ones
The per-core partials are AllReduce-summed across the 8 cores, then every
core applies the tiny [86,256] update and computes the scalar loss.
"""

import os
import sys

for _p in (
    "/root/.axon_site",
    "/root/.axon_site/_ro/trn_rl_repo",
    "/root/.axon_site/_ro/pypackages",
    "/opt/trn_rl_repo",
    "/opt/pypackages",
):
    if os.path.isdir(_p) and _p not in sys.path:
        sys.path.append(_p)

import numpy as np

ALPHA = 0.5
C = 86          # num classes
F = 256         # feature dim
N = 131072      # total samples
NCORES = 8
NS = N // NCORES          # samples per core = 16384
P = 128                   # partitions
TILES = NS // P           # 128 sample-tiles per core
T = 16                    # sample-tiles per super-tile (DMA batch)
NSUPER = TILES // T       # 8
FW = F + 2                # 258: payload row = [S_c | count_c | sumsq-slot]


def build_nc(feat_bufs=3, unroll=1, serialize=True, legalize=True):
    import concourse.bass as bass
    import concourse.mybir as mybir
    from concourse import tile
    from concourse.masks import make_identity

    f32 = mybir.dt.float32
    i32 = mybir.dt.int32

    nc = bass.Bass(
        "TRN2",
        target_bir_lowering=False,
        debug=False,
        num_devices=NCORES,
    )

    feat = nc.dram_tensor("features", [NS, F], f32, kind="ExternalInput").ap()
    lab = nc.dram_tensor("labels", [NS], i32, kind="ExternalInput").ap()
    cent = nc.dram_tensor("centers", [C, F], f32, kind="ExternalInput").ap()
    out_c = nc.dram_tensor("out_centers", [C, F], f32, kind="ExternalOutput").ap()
    out_l = nc.dram_tensor("out_loss", [1, 1], f32, kind="ExternalOutput").ap()

    with tile.TileContext(nc) as tc:
        with (
            tc.tile_pool(name="const", bufs=1) as const_pool,
            tc.tile_pool(name="lprep", bufs=1) as lprep_pool,
            tc.tile_pool(name="feat", bufs=feat_bufs) as feat_pool,
            tc.tile_pool(name="oh", bufs=6) as oh_pool,
            tc.tile_pool(name="sq", bufs=4) as sq_pool,
            tc.tile_pool(name="fin", bufs=1) as fin_pool,
            tc.tile_pool(name="psum", bufs=1, space="PSUM") as psum_pool,
            tc.tile_pool(name="psum2", bufs=1, space="PSUM") as psum2_pool,
            tc.tile_pool(name="dram", bufs=1, space="DRAM") as dram_pool,
        ):
            # ---- compile-time constants (shared across bench iterations) ----
            ident = const_pool.tile([P, P], f32)
            make_identity(nc, ident[:, :])
            iota_i = const_pool.tile([P, C], i32)
            nc.gpsimd.iota(iota_i[:, :], pattern=[[1, C]], base=0,
                           channel_multiplier=0)
            iota_f = const_pool.tile([P, C], f32)
            nc.vector.tensor_copy(iota_f[:, :], iota_i[:, :])
            ones_col = const_pool.tile([P, 1], f32)
            nc.vector.memset(ones_col[:, :], 1.0)

            prev_cc_out = None
            for it in range(unroll):
                prev_cc_out = _one_pass(
                    nc, tc, mybir, feat, lab, cent, out_c, out_l,
                    ident, iota_f, ones_col,
                    lprep_pool, feat_pool, oh_pool, sq_pool, fin_pool,
                    psum_pool, psum2_pool, dram_pool,
                    prev_cc_out if serialize else None)

    if legalize:
        _legalize(nc)
    return nc


def _legalize(nc):
    """Make the module digestible by this container's (older) walrus.

    1. Remove kernel-exit EVENT_SEMAPHORE_RANGE_CLEAR InstISA ops: walrus
       rejects their encoding ("ISA wrong length") and they only pre-clear
       semaphores for a subsequent bass kernel in the same NEFF, which we
       never emit.
    2. Engine instruction structs here accept a single inline sync wait
       ("Too many sync wait commands" otherwise). Hoist all but one wait of
       any multi-wait instruction onto wait-only EventSemaphore instructions
       issued just before it on the same engine.
    """
    from concourse import mybir

    removed = set()
    for f in nc.m.functions:
        for b in f.blocks:
            keep = []
            for inst in b.instructions:
                if type(inst).__name__ == "InstISA":
                    removed.add(inst.name)
                else:
                    keep.append(inst)
            b.instructions[:] = keep
    if removed:
        for f in nc.m.functions:
            for b in f.blocks:
                for inst in b.instructions:
                    deps = list(inst.sync_dependencies())
                    if any(d in removed for d, _ in deps):
                        inst.set_sync_dependencies(
                            [(d, i) for d, i in deps if d not in removed])

    for f in nc.m.functions:
        for b in f.blocks:
            out = []
            for inst in b.instructions:
                si = inst.sync_info
                if si is not None and si.on_wait and len(si.on_wait) > 1:
                    waits = list(si.on_wait)
                    for w in waits[:-1]:
                        ev = mybir.InstEventSemaphore(
                            name=f"{inst.name}_hw{w.id}", ins=[], outs=[])
                        ev.engine = inst.engine
                        ev.sync_info = mybir.SyncInfo(
                            on_wait=[w], on_update=[])
                        out.append(ev)
                    inst.sync_info = mybir.SyncInfo(
                        on_wait=[waits[-1]], on_update=list(si.on_update))
                out.append(inst)
            b.instructions[:] = out


def _one_pass(nc, tc, mybir, feat, lab, cent, out_c, out_l,
              ident, iota_f, ones_col,
              lprep_pool, feat_pool, oh_pool, sq_pool, fin_pool,
              psum_pool, psum2_pool, dram_pool, prev_cc_out):
    f32 = mybir.dt.float32
    i32 = mybir.dt.int32

    # ---- label prep: [16384] -> transposed [128 x 128] f32 so that
    # labT[:, i] holds the 128 labels of sample-tile i ----
    # (convert on gpsimd so the transpose waits on a single Pool sem:
    # matmuls only support one sync wait in this codegen)
    lab2d = lab.rearrange("(p n) -> p n", p=P)
    lab_i = lprep_pool.tile([P, TILES], i32)
    nc.sync.dma_start(out=lab_i[:, :], in_=lab2d)
    lab_f = lprep_pool.tile([P, TILES], f32)
    nc.gpsimd.tensor_copy(lab_f[:, :], lab_i[:, :])
    labT_ps = psum2_pool.tile([P, TILES], f32, tag="ps2")
    nc.tensor.transpose(labT_ps[:, :], lab_f[:, :], ident[:, :])
    labT = lprep_pool.tile([P, TILES], f32)
    nc.scalar.copy(labT[:, :], labT_ps[:, :])

    # centers for the finalize stage
    ct = lprep_pool.tile([C, F], f32)
    nc.sync.dma_start(out=ct[:, :], in_=cent)

    # ---- streaming one-hot matmul accumulation ----
    # feature tiles carry a ones column (memset once per supertile on the
    # idle Pool engine): the single matmul per tile then yields both the
    # per-class feature sums AND the counts in psum [86, 257]. The
    # supertile-wide ACT square accumulates sum(f^2) + 16 (ones cols) per
    # partition; the spurious +N is subtracted at finalize.
    FTW = F + 1
    acc = psum_pool.tile([C, FTW], f32, tag="acc")
    sqbuf = lprep_pool.tile([P, NSUPER], f32)

    for s in range(NSUPER):
        ft = feat_pool.tile([P, T * FTW], f32)
        if prev_cc_out is not None:
            # benchmarking only: chain this pass's loads on the previous
            # pass's collective so passes don't overlap
            flat = prev_cc_out[:, :].rearrange("c f -> (c f)")
            nc.sync.dma_start(
                out=ft[:, 0:1],
                in_=flat[s * P:(s + 1) * P].rearrange("(p o) -> p o", o=1))
        fv = ft[:, :].rearrange("p (t f) -> p t f", f=FTW)
        src = feat[s * T * P:(s + 1) * T * P, :].rearrange(
            "(t p) f -> p t f", p=P)
        nc.sync.dma_start(out=fv[:, :, 0:F], in_=src)
        nc.gpsimd.memset(fv[:, :, F:FTW], 1.0)
        sqo = sq_pool.tile([P, T * FTW], f32, tag="sqo")
        nc.scalar.activation(
            out=sqo[:, :], in_=ft[:, :],
            func=mybir.ActivationFunctionType.Square,
            accum_out=sqbuf[:, s:s + 1])
        for t in range(T):
            i = s * T + t
            oh = oh_pool.tile([P, C], f32)
            nc.vector.tensor_single_scalar(
                out=oh[:, :], in_=iota_f[:, :],
                scalar=labT[:, i:i + 1],
                op=mybir.AluOpType.is_equal)
            nc.tensor.matmul(
                acc[:, :], lhsT=oh[:, :], rhs=ft[:, t * FTW:(t + 1) * FTW],
                start=(i == 0), stop=(i == TILES - 1))

    # ---- reduce the sumsq accumulator with one matmul ----
    sqr = fin_pool.tile([P, 1], f32, tag="sqr")
    nc.vector.reduce_sum(sqr[:, :], sqbuf[:, :], axis=mybir.AxisListType.X)
    sq_ps = psum_pool.tile([1, 1], f32, tag="sq_ps")
    nc.tensor.matmul(sq_ps[:, :], lhsT=sqr[:, :], rhs=ones_col[:, :],
                     start=True, stop=True)

    # ---- pack partials [S | counts | sumsq@row0]; AllGather + local sum
    # (AG floor ~5us vs AR ~18us at this size) ----
    part = fin_pool.tile([C, FW], f32, tag="part")
    nc.vector.tensor_copy(part[:, 0:FTW], acc[:, :])
    nc.vector.memset(part[0:C, F + 1:FW], 0.0)
    nc.vector.tensor_copy(part[0:1, F + 1:FW], sq_ps[:, :])

    cc_in = dram_pool.tile([C, FW], f32)
    cc_out = dram_pool.tile([NCORES * C, FW], f32)
    nc.sync.dma_start(out=cc_in[:, :], in_=part[:, :])
    nc.gpsimd.collective_compute(
        "AllGather",
        mybir.AluOpType.bypass,
        replica_groups=[list(range(NCORES))],
        ins=[cc_in[:, :].opt()],
        outs=[cc_out[:, :].opt()],
    )
    agbuf = fin_pool.tile([C, NCORES * FW], f32, tag="agbuf")
    agv = agbuf[:, :].rearrange("c (r f) -> c r f", f=FW)
    nc.sync.dma_start(
        out=agv[:, :, :],
        in_=cc_out[:, :].rearrange("(r c) f -> c r f", c=C))
    allsum = fin_pool.tile([C, FW], f32, tag="allsum")
    nc.vector.tensor_add(allsum[:, :], agv[:, 0, :], agv[:, 1, :])
    for r in range(2, NCORES):
        nc.vector.tensor_add(allsum[:, :], allsum[:, :], agv[:, r, :])

    # ---- finalize: new_centers = a*centers + r*S ----
    S = allsum[0:C, 0:F]
    cnt = allsum[0:C, F:F + 1]
    sqt_t = allsum  # [0:1, F+1:FW] holds the global sum of squares

    cnt1 = fin_pool.tile([C, 1], f32, tag="cnt1")
    nc.vector.tensor_scalar_add(cnt1[:, :], cnt, 1.0)
    rec = fin_pool.tile([C, 1], f32, tag="rec")
    nc.vector.reciprocal(rec[:, :], cnt1[:, :])
    r = fin_pool.tile([C, 1], f32, tag="r")
    nc.vector.tensor_scalar_mul(r[:, :], rec[:, :], ALPHA)
    rcnt = fin_pool.tile([C, 1], f32, tag="rcnt")
    nc.vector.tensor_mul(rcnt[:, :], r[:, :], cnt)
    a = fin_pool.tile([C, 1], f32, tag="a")
    nc.vector.tensor_scalar(
        out=a[:, :], in0=rcnt[:, :], scalar1=-1.0, scalar2=1.0,
        op0=mybir.AluOpType.mult, op1=mybir.AluOpType.add)

    newc = fin_pool.tile([C, F], f32, tag="newc")
    nc.vector.tensor_single_scalar(
        out=newc[:, :], in_=ct[:, :], scalar=a[:, 0:1],
        op=mybir.AluOpType.mult)
    sterm = fin_pool.tile([C, F], f32, tag="sterm")
    nc.vector.tensor_single_scalar(
        out=sterm[:, :], in_=S, scalar=r[:, 0:1],
        op=mybir.AluOpType.mult)
    newc2 = fin_pool.tile([C, F], f32, tag="newc2")
    nc.vector.tensor_add(newc2[:, :], newc[:, :], sterm[:, :])
    nc.sync.dma_start(out=out_c, in_=newc2[:, :])

    # ---- loss = 0.5*sumsq - sum_c dot(S,c) + 0.5*sum_c cnt*|c|^2 ----
    g = fin_pool.tile([C, 1], f32, tag="g")
    dots = fin_pool.tile([C, 1], f32, tag="dots")
    scr1 = fin_pool.tile([C, F], f32, tag="scr1")
    nc.vector.tensor_mul(scr1[:, :], S, ct[:, :])
    nc.vector.reduce_sum(dots[:, :], scr1[:, :], axis=mybir.AxisListType.X)
    c2 = fin_pool.tile([C, 1], f32, tag="c2")
    scr2 = fin_pool.tile([C, F], f32, tag="scr2")
    nc.vector.tensor_mul(scr2[:, :], ct[:, :], ct[:, :])
    nc.vector.reduce_sum(c2[:, :], scr2[:, :], axis=mybir.AxisListType.X)
    h = fin_pool.tile([C, 1], f32, tag="h")
    nc.vector.tensor_mul(h[:, :], c2[:, :], cnt)
    hm = fin_pool.tile([C, 1], f32, tag="hm")
    nc.vector.tensor_scalar_mul(hm[:, :], h[:, :], 0.5)
    nc.vector.tensor_sub(g[:, :], hm[:, :], dots[:, :])

    ones_c1 = fin_pool.tile([C, 1], f32, tag="ones_c1")
    nc.vector.memset(ones_c1[:, :], 1.0)
    lps = psum2_pool.tile([1, 1], f32, tag="ps2")
    nc.tensor.matmul(lps[:, :], lhsT=g[:, :], rhs=ones_c1[:, :],
                     start=True, stop=True)
    sqh = fin_pool.tile([1, 1], f32, tag="sqh")
    nc.vector.tensor_scalar(
        out=sqh[:, :], in0=sqt_t[0:1, F + 1:FW], scalar1=-float(N),
        scalar2=0.5, op0=mybir.AluOpType.add, op1=mybir.AluOpType.mult)
    lsb = fin_pool.tile([1, 1], f32, tag="lsb")
    nc.vector.tensor_add(lsb[:, :], lps[0:1, 0:1], sqh[:, :])
    nc.sync.dma_start(out=out_l, in_=lsb[:, :])

    return cc_out


_NC_CACHE = None
LAST_RESULT = None  # stashed BassKernelResults for test.py introspection


def _get_nc():
    global _NC_CACHE
    if _NC_CACHE is None:
        _NC_CACHE = build_nc()
    return _NC_CACHE


def kernel(features, labels, centers):
    global LAST_RESULT
    from concourse.bass_utils import run_bass_kernel_spmd

    features = np.ascontiguousarray(np.asarray(features, dtype=np.float32))
    labels = np.asarray(labels).astype(np.int32).ravel()
    centers = np.ascontiguousarray(np.asarray(centers, dtype=np.float32))

    nc = _get_nc()
    in_maps = [
        {
            "features": features[r * NS:(r + 1) * NS],
            "labels": labels[r * NS:(r + 1) * NS],
            "centers": centers,
        }
        for r in range(NCORES)
    ]
    trace = os.environ.get("KERNEL_TRACE", "0") == "1"
    res = run_bass_kernel_spmd(
        nc, in_maps, core_ids=list(range(NCORES)), trace=trace)
    LAST_RESULT = res
    out = res.results[0]
    loss = np.float32(np.asarray(out["out_loss"]).reshape(-1)[0])
    new_centers = np.asarray(out["out_centers"], dtype=np.float32).reshape(C, F)
    return loss, new_centers
